# revision 41
# baseline (speedup 1.0000x reference)
"""Multi-head attention kernel for 8 Trainium2 NeuronCores.

Problem: B=4, S=2048, D=H=1024, NH=16 heads (head_dim 64), causal MHA with
input projections (W_q/W_k/W_v), softmax, and output projection (W_o).

Sharding: 8 cores = 4 batches x 2 head-groups (tensor parallel over heads).
Each core computes, for one batch b and one group g of 8 heads (4 head-pairs):
  QT = (x W_q[g].T + b_q).T   [feature, seq]  (b_k dropped: its effect on
  KT = (x W_k[g].T).T                          softmax is a per-query constant)
  V  = x W_v[g].T             [seq, feature]  (b_v folded on host)
  per head-pair hp, per 512-wide q chunk, per 128-wide k block:
    S.T = KT_h.T QT_h  (zero-padded K strips -> full 128-row matmuls)
    P.T = exp(S.T/8 + causal) for both strips in one ACT instruction
    O.T += V_aug_h.T @ P.T  (ones column -> row 64 = softmax denominator)
  All score/exp/PV work is trimmed to the causal region at column granularity.
  partial.T = W_o[:, g].T-slice contracted with (O.T / denom)  (row-parallel)
The host sums the two group partials per batch, transposes, and adds
(W_o @ b_v + b_o) exactly.

Everything runs in bf16 operands (f32 PSUM accumulation): rel-err budget is
2e-2 and bf16 measures 4.8e-3 on HW. Scheduling: the V and Q/K projections
for later head-pairs are emitted as generators interleaved into the earlier
head-pairs' attention loops (and the output projection into the last one),
so the PE's projection matmuls fill the per-block gap left while the ACT
engine (exp) paces attention; causal trimming at column granularity plus the
0/1-triangle mask applied to P on the otherwise-idle GPSIMD engine keeps
ACT/DVE off the critical path.
"""

import os
import sys

if "/opt/trn_rl_repo" not in sys.path:
    sys.path.insert(0, "/opt/trn_rl_repo")

import numpy as np
import ml_dtypes

import concourse.mybir as mybir
import concourse.tile as tile
from concourse import bacc
from concourse.bass_utils import run_bass_kernel_spmd

F32 = mybir.dt.float32
BF16 = mybir.dt.bfloat16
F8 = mybir.dt.float8e4
EXP = mybir.ActivationFunctionType.Exp
CPY = mybir.ActivationFunctionType.Copy
IDN = mybir.ActivationFunctionType.Identity
DR = mybir.MatmulPerfMode.DoubleRow

# fp8 (DoubleRow, 2x PE) Q/K projections: measured rel err 3.5e-2 on HW —
# over the 2e-2 gate, so OFF. (bf16 everywhere measures 4.8e-3.)
QK_FP8 = os.environ.get("QK_FP8", "0") == "1"
WSCALE = 64.0

# Problem dims (full) and per-core dims
B, S, D, H, NH, HD = 4, 2048, 1024, 1024, 16, 64
HL = H // 2          # per-core feature width (8 heads x 64)
NHL = HL // HD       # 8 local heads
D_TILES = D // 128   # 8
J_TILES = HL // 128  # 4 head-pairs
S_TILES = S // 128   # 16
QC = S // 512        # 4 q-chunks (512 wide)
NEG = -1.0e30
PIPE = 8             # PV matmuls trail the score/exp pipeline by this many

_nc_cache = {}


def _build_nc(upto="all", reps=1):
    key = (upto, reps, QK_FP8)
    if key in _nc_cache:
        return _nc_cache[key]

    nc = bacc.Bacc("TRN2", target_bir_lowering=False, debug=False)

    QKDT = F8 if QK_FP8 else BF16
    xq_t = nc.dram_tensor("xq_t", [D, S], QKDT, kind="ExternalInput")
    xk_t = nc.dram_tensor("xk_t", [D, S], QKDT, kind="ExternalInput")
    xv_t = nc.dram_tensor("xv_t", [D, S], BF16, kind="ExternalInput")
    wq_t = nc.dram_tensor("wq_t", [D, HL], QKDT, kind="ExternalInput")
    wk_t = nc.dram_tensor("wk_t", [D, HL], QKDT, kind="ExternalInput")
    wv_t = nc.dram_tensor("wv_t", [D, HL], BF16, kind="ExternalInput")
    wo_t = nc.dram_tensor("wo_t", [HL, D], BF16, kind="ExternalInput")
    bq = nc.dram_tensor("bq", [HL], F32, kind="ExternalInput")
    masks = nc.dram_tensor("masks", [128, 2, 128], BF16, kind="ExternalInput")
    partial_t = nc.dram_tensor("partial_t", [D, S], BF16, kind="ExternalOutput")
    rr_dram = nc.dram_tensor("rr_scratch", [NHL, S], BF16, kind="Internal")

    with tile.TileContext(nc) as tc:
        with tc.tile_pool(name="persist", bufs=1) as pp:
            mask_sb = pp.tile([128, 2, 128], BF16, tag="masks")
            bq_sb = pp.tile([128, J_TILES], F32, tag="bq")
            wo_sb = pp.tile([128, J_TILES, D], BF16, tag="wo")

            QT = [pp.tile([128, S], BF16, tag=f"qt{j}", name=f"qt{j}")
                  for j in range(J_TILES)]
            # zero-padded K strip copies: KT[j][0] has head-strip 0 rows
            # (0:64) live and rows 64:128 zero, KT[j][1] the reverse, so
            # score matmuls contract the dead rows against zeros.
            KT = [[pp.tile([128, S], BF16, tag=f"kt{j}_{s}", name=f"kt{j}_{s}")
                   for s in range(2)] for j in range(J_TILES)]
            V = [pp.tile([128, NHL, HD + 1], BF16, tag=f"v{st}", name=f"v{st}")
                 for st in range(S_TILES)]
            OT = [pp.tile([128, S], BF16, tag=f"ot{j}", name=f"ot{j}")
                  for j in range(J_TILES)]
            RBD = pp.tile([128, S], BF16, tag="rbd")
            RB = pp.tile([128, S], F32, tag="rb")

            for _rep in range(reps):
                _emit_pipeline(nc, tc, upto, mask_sb, bq_sb, wo_sb, QT, KT, V,
                               OT, RB, RBD, rr_dram, xq_t, xk_t, xv_t, wq_t,
                               wk_t, wv_t, wo_t, partial_t, masks, bq)

    nc.finalize()
    _nc_cache[key] = nc
    return nc


def _emit_pipeline(nc, tc, upto, mask_sb, bq_sb, wo_sb, QT, KT, V, OT, RB,
                   RBD, rr_dram, xq_t, xk_t, xv_t, wq_t, wk_t, wv_t, wo_t,
                   partial_t, masks_d, bq_d):
    from contextlib import ExitStack

    # zero the dead K strip halves (idle GPSIMD engine)
    for j in range(J_TILES):
        nc.gpsimd.memset(KT[j][0][64:128, :], 0.0)
        nc.gpsimd.memset(KT[j][1][0:64, :], 0.0)
    for st in range(S_TILES):
        nc.gpsimd.memset(V[st][:, :, HD], 1.0)  # softmax-denominator column

    import itertools

    # attention pools open first (outermost) so the projection pools can be
    # released mid-stream (pool release must be LIFO)
    actx = ExitStack()
    with actx:
        scp = actx.enter_context(
            tc.tile_pool(name="scp", bufs=1, space="PSUM"))
        pvp = actx.enter_context(
            tc.tile_pool(name="pvp", bufs=1, space="PSUM"))
        ptp = actx.enter_context(tc.tile_pool(name="ptp", bufs=1))
        stg = actx.enter_context(tc.tile_pool(name="stg", bufs=1))
        _emit_inner(nc, tc, upto, mask_sb, bq_sb, wo_sb, QT, KT, V, OT, RB,
                    RBD, rr_dram, xq_t, xk_t, xv_t, wq_t, wk_t, wv_t, wo_t,
                    partial_t, scp, pvp, ptp, stg, masks_d, bq_d)


def _emit_inner(nc, tc, upto, mask_sb, bq_sb, wo_sb, QT, KT, V, OT, RB,
                RBD, rr_dram, xq_t, xk_t, xv_t, wq_t, wk_t, wv_t, wo_t,
                partial_t, scp, pvp, ptp, stg, masks_d, bq_d):
    from contextlib import ExitStack
    import itertools

    ctx = ExitStack()
    with ctx:
        qkw = ctx.enter_context(tc.tile_pool(name="qkw", bufs=1))
        qkx = ctx.enter_context(tc.tile_pool(name="qkx", bufs=1))
        # shared by the V-projection and Q/K-projection chains
        qkp = ctx.enter_context(tc.tile_pool(name="qkp", bufs=2, space="PSUM"))

        wv_sb = qkw.tile([128, D_TILES, HL], BF16, tag="wv", name="wv")
        wv_loaded = [False]

        def v_feeder(st_from, st_to, on_act=False):
            for st in range(st_from, st_to):
                ssl = slice(st * 128, (st + 1) * 128)
                xv_blk = qkx.tile([128, D_TILES, 128], BF16, tag="xvb",
                                  name="xvb", bufs=3)
                nc.sync.dma_start(
                    out=xv_blk,
                    in_=xv_t[:, ssl].rearrange("(dt p) f -> p dt f", p=128),
                )
                if not wv_loaded[0]:
                    # issue behind the first xv block; split so the chain's
                    # first matmuls only wait for the first half
                    wv_loaded[0] = True
                    for h in range(2):
                        nc.sync.dma_start(
                            out=wv_sb[:, 4 * h:4 * h + 4, :],
                            in_=wv_t[512 * h:512 * h + 512, :]
                            .rearrange("(dt p) j -> p dt j", p=128),
                        )
                yield
                ps = qkp.tile([128, HL], F32, tag="pp")
                for dt in range(D_TILES):
                    nc.tensor.matmul(
                        ps, xv_blk[:, dt, :], wv_sb[:, dt, :],
                        start=(dt == 0), stop=(dt == D_TILES - 1),
                    )
                if on_act:
                    nc.scalar.activation(
                        V[st][:, :, 0:HD],
                        ps[:].rearrange("p (h x) -> p h x", h=NHL), CPY)
                else:
                    nc.vector.tensor_copy(
                        V[st][:, :, 0:HD],
                        ps[:].rearrange("p (h x) -> p h x", h=NHL)
                    )
                yield

        def qk_feeder(jts, on_act=False):
            """Emit the Q/K projections for the given head-pairs, yielding
            between chunks so attention emission can interleave."""
            jw = 128 * len(jts)
            jsl_dram = slice(jts[0] * 128, jts[0] * 128 + jw)
            qkdt = F8 if QK_FP8 else BF16
            for (w_dram, x_dram, is_k, wtag) in (
                (wq_t, xq_t, False, "wq"),
                (wk_t, xk_t, True, "wk"),
            ):
                w_sb = qkw.tile([128, D_TILES, jw], qkdt,
                                tag=f"{wtag}{len(jts)}", name=wtag, bufs=2)
                nc.sync.dma_start(
                    out=w_sb,
                    in_=w_dram[:, jsl_dram].rearrange("(dt p) j -> p dt j",
                                                      p=128),
                )
                for sc in range(4):
                    scsl = slice(sc * 512, (sc + 1) * 512)
                    xblks = []
                    for half in range(2):
                        xb = qkx.tile([128, 4, 512], qkdt, tag=f"xb{half}",
                                      name=f"xb{half}", bufs=2)
                        nc.sync.dma_start(
                            out=xb,
                            in_=x_dram[half * 512:(half + 1) * 512, scsl]
                            .rearrange("(dt p) f -> p dt f", p=128),
                        )
                        xblks.append(xb)
                    yield
                    for jloc, jt in enumerate(jts):
                        jsl = slice(jloc * 128, (jloc + 1) * 128)
                        ps = qkp.tile([128, 512], F32, tag="pp")
                        if QK_FP8:
                            for i in range(4):  # dt pairs, DoubleRow
                                nc.tensor.matmul(
                                    ps, w_sb[:, 2 * i:2 * i + 2, jsl],
                                    xblks[i // 2][:, 2 * (i % 2):
                                                  2 * (i % 2) + 2, :],
                                    start=(i == 0), stop=(i == 3),
                                    perf_mode=DR,
                                )
                        else:
                            for dt in range(D_TILES):
                                nc.tensor.matmul(
                                    ps, w_sb[:, dt, jsl],
                                    xblks[dt // 4][:, dt % 4, :],
                                    start=(dt == 0), stop=(dt == D_TILES - 1),
                                )
                        if is_k:
                            if on_act:
                                nc.scalar.activation(KT[jt][0][0:64, scsl],
                                                     ps[0:64, :], CPY)
                                nc.scalar.activation(KT[jt][1][64:128, scsl],
                                                     ps[64:128, :], CPY)
                            else:
                                nc.vector.tensor_copy(KT[jt][0][0:64, scsl],
                                                      ps[0:64, :])
                                nc.vector.tensor_copy(KT[jt][1][64:128, scsl],
                                                      ps[64:128, :])
                        elif on_act:
                            nc.scalar.activation(QT[jt][:, scsl], ps, IDN,
                                                 bias=bq_sb[:, jt:jt + 1])
                        else:
                            nc.vector.tensor_scalar_add(
                                QT[jt][:, scsl], ps, bq_sb[:, jt:jt + 1]
                            )
                        yield

        # prologue: V tiles 0..3, then Q/K for head-pair 0 only (PSUM
        # drains on the ACT engine, which would otherwise idle until
        # attention); head-pair 1's projections stream into the attention
        # loop with everything else
        for _ in v_feeder(0, 4, on_act=True):
            pass
        # consts issue behind the first V loads: SP serializes DMA issue at
        # ~650ns each, and these aren't needed until attention starts
        nc.sync.dma_start(out=mask_sb, in_=masks_d[:])
        nc.sync.dma_start(
            out=bq_sb, in_=bq_d[:].rearrange("(t p) -> p t", p=128))
        for _ in qk_feeder((0, 1), on_act=True):
            pass
        nc.sync.dma_start(
            out=wo_sb, in_=wo_t[:].rearrange("(jt p) d -> p jt d", p=128)
        )
        if upto == "proj":
            for _ in v_feeder(4, S_TILES):
                pass
            for _ in qk_feeder((2, 3)):
                pass
            return

        # ================= attention =================
        def attention(hp, feeder, qi_end=None, qis=None):
            for Qi in (range(QC) if qis is None else qis):
                q0 = Qi * 512
                qsl = slice(q0, q0 + 512)
                nk = 4 * (Qi + 1)
                pv = [pvp.tile([HD + 1, 512], F32, tag=f"pv{s}",
                               name=f"pv{s}") for s in range(2)]
                pending = []

                def flush_one():
                    pt_, off_, ki_ = pending.pop(0)
                    for s in range(2):
                        nc.tensor.matmul(
                            pv[s][:, off_:512], V[ki_][:, 2 * hp + s, :],
                            pt_[:, s, off_:512],
                            start=(ki_ == 0), stop=(ki_ == nk - 1),
                            skip_group_check=True,
                        )

                for ki in range(nk):
                    k0 = ki * 128
                    off = max(0, k0 - q0)
                    st = scp.tile([128, 2, 512], F32, tag="st",
                                  name="st", bufs=2)
                    for s in range(2):
                        nc.tensor.matmul(
                            st[:, s, off:512], KT[hp][s][:, k0:k0 + 128],
                            QT[hp][:, q0 + off:q0 + 512],
                            start=True, stop=True,
                        )
                    pt = ptp.tile([128, 2, 512], BF16, tag="pt",
                                  name="pt", bufs=PIPE + 2)
                    escale = 0.125 / (WSCALE * WSCALE) if QK_FP8 else 0.125
                    nc.scalar.activation(pt[:, :, off:512],
                                         st[:, :, off:512], EXP,
                                         scale=escale)
                    if k0 >= q0:  # diagonal block: 0/1 triangle on idle Q7
                        nc.gpsimd.tensor_mul(pt[:, :, off:off + 128],
                                             pt[:, :, off:off + 128],
                                             mask_sb)
                    pending.append((pt, off, ki))
                    if len(pending) > PIPE:
                        flush_one()
                    if feeder is not None:
                        next(feeder, None)
                while pending:
                    flush_one()
                for s in range(2):
                    stage = stg.tile([HD + 1, 512], BF16, tag="stg",
                                     name="stage", bufs=3)
                    nc.vector.tensor_copy(stage, pv[s][:, :])
                    nc.sync.dma_start(
                        out=OT[hp][s * 64:(s + 1) * 64, qsl],
                        in_=stage[0:64, :])
                    # denominator row round-trips through DRAM so the DMA
                    # engine partition-broadcasts it into RBD
                    nc.sync.dma_start(
                        out=rr_dram[2 * hp + s, qsl], in_=stage[64:65, :])
                    nc.sync.dma_start(
                        out=RBD[s * 64:(s + 1) * 64, qsl],
                        in_=rr_dram[2 * hp + s:2 * hp + s + 1, qsl]
                        .to_broadcast((64, 512)))
                nc.vector.reciprocal(RB[:, qsl], RBD[:, qsl])
                nc.vector.tensor_mul(OT[hp][:, qsl], OT[hp][:, qsl],
                                     RB[:, qsl])
                if qi_end is not None:
                    qi_end(Qi)

        def paced(gen, period):
            for item in gen:
                yield item
                for _ in range(period - 1):
                    yield None

        # pace the later projections so they also soak up head-pair 2's
        # PE slack
        feeder = itertools.chain(v_feeder(4, S_TILES),
                                 paced(qk_feeder((2, 3)), 3 if QK_FP8 else 2))
        attention(0, feeder)
        attention(1, feeder)
        attention(2, feeder)
        for _ in feeder:  # make sure wave-1 projections are fully emitted
            pass
    # qk pools released here (SBUF + 2 PSUM banks)

    with (
        tc.tile_pool(name="opp", bufs=2, space="PSUM") as opp,
        tc.tile_pool(name="ost", bufs=1) as ost,
    ):
        def outproj_sc(sc):
            scsl = slice(sc * 512, (sc + 1) * 512)
            for dt in range(D_TILES):
                dsl = slice(dt * 128, (dt + 1) * 128)
                ps = opp.tile([128, 512], F32, tag="op")
                for jt in range(J_TILES):
                    nc.tensor.matmul(
                        ps, wo_sb[:, jt, dsl], OT[jt][:, scsl],
                        start=(jt == 0), stop=(jt == J_TILES - 1),
                    )
                ostage = ost.tile([128, 512], BF16, tag="ostage",
                                  name="ostage", bufs=3)
                nc.vector.tensor_copy(ostage, ps)
                nc.sync.dma_start(out=partial_t[dsl, scsl], in_=ostage)
                yield

        # interleave the output projection of q-chunk Qi-1 into head-pair
        # 3's attention on q-chunk Qi
        oproj = [None]

        def hp3_qi_end(Qi):
            if oproj[0] is not None:
                for _ in oproj[0]:
                    pass
            oproj[0] = outproj_sc(Qi)

        def hp3_feeder():
            while True:
                if oproj[0] is not None:
                    next(oproj[0], None)
                yield

        # descending q-chunks: the final chunk (Qi=0) is the shortest, so
        # the last normalize -> outproj tail is as small as possible
        attention(3, hp3_feeder(), qi_end=hp3_qi_end, qis=(3, 2, 1, 0))
        for _ in oproj[0]:
            pass


def _make_masks():
    p = np.arange(128)[:, None]
    f = np.arange(128)[None, :]
    tri = np.where(p > f, 0.0, 1.0).astype(ml_dtypes.bfloat16)
    return np.broadcast_to(tri[:, None, :], (128, 2, 128)).copy()


def _make_in_maps(q, k, v, W_q, b_q, W_k, b_k, W_v, b_v, W_o):
    bf = ml_dtypes.bfloat16
    qkdt = ml_dtypes.float8_e4m3 if QK_FP8 else bf
    ws = WSCALE if QK_FP8 else 1.0
    masks = _make_masks()
    in_maps = []
    for core in range(8):
        b, g = divmod(core, 2)
        gsl = slice(g * HL, (g + 1) * HL)
        in_maps.append({
            "xq_t": np.ascontiguousarray(q[b].T).astype(qkdt),
            "xk_t": np.ascontiguousarray(k[b].T).astype(qkdt),
            "xv_t": np.ascontiguousarray(v[b].T).astype(bf),
            "wq_t": np.ascontiguousarray(W_q[gsl].T * ws).astype(qkdt),
            "wk_t": np.ascontiguousarray(W_k[gsl].T * ws).astype(qkdt),
            "wv_t": np.ascontiguousarray(W_v[gsl].T).astype(bf),
            "wo_t": np.ascontiguousarray(W_o[:, gsl].T).astype(bf),
            "bq": np.ascontiguousarray(b_q[gsl] * ws).astype(np.float32),
            "masks": masks,
        })
    return in_maps


def kernel(q, k, v, padding_mask, W_q, b_q, W_k, b_k, W_v, b_v, W_o, b_o):
    q = np.asarray(q, np.float32)
    k = np.asarray(k, np.float32)
    v = np.asarray(v, np.float32)
    W_q = np.asarray(W_q, np.float32)
    W_k = np.asarray(W_k, np.float32)
    W_v = np.asarray(W_v, np.float32)
    W_o = np.asarray(W_o, np.float32)
    b_q = np.asarray(b_q, np.float32)
    b_k = np.asarray(b_k, np.float32)
    b_v = np.asarray(b_v, np.float32)
    b_o = np.asarray(b_o, np.float32)
    padding_mask = np.asarray(padding_mask)

    if padding_mask.any():
        return _numpy_reference(q, k, v, padding_mask, W_q, b_q, W_k, b_k,
                                W_v, b_v, W_o, b_o)

    nc = _build_nc()
    in_maps = _make_in_maps(q, k, v, W_q, b_q, W_k, b_k, W_v, b_v, W_o)

    res = run_bass_kernel_spmd(nc, in_maps, core_ids=list(range(8)))

    bias_vec = (W_o @ b_v + b_o).astype(np.float32)  # exact v/out bias folding
    out = np.empty((B, S, D), np.float32)
    for b in range(B):
        pt = (res.results[2 * b]["partial_t"].astype(np.float32)
              + res.results[2 * b + 1]["partial_t"].astype(np.float32))
        out[b] = pt.T + bias_vec
    return out


def _numpy_reference(q, k, v, padding_mask, W_q, b_q, W_k, b_k, W_v, b_v,
                     W_o, b_o):
    """Slow exact path, only used when padding_mask is nonzero."""
    Q = (q @ W_q.T + b_q).reshape(B, S, NH, HD).transpose(0, 2, 1, 3)
    K = (k @ W_k.T + b_k).reshape(B, S, NH, HD).transpose(0, 2, 1, 3)
    Vv = (v @ W_v.T + b_v).reshape(B, S, NH, HD).transpose(0, 2, 1, 3)
    scores = np.einsum("bhqd,bhkd->bhqk", Q, K) / np.sqrt(HD)
    causal = np.triu(np.ones((S, S), bool), k=1)
    scores = np.where(causal, -np.inf, scores)
    scores = np.where(padding_mask[:, None, None, :], -np.inf, scores)
    scores = scores - scores.max(axis=-1, keepdims=True)
    e = np.exp(scores)
    attn = e / e.sum(axis=-1, keepdims=True)
    out = np.einsum("bhqk,bhkd->bhqd", attn, Vv)
    out = out.transpose(0, 2, 1, 3).reshape(B, S, H)
    return out @ W_o.T + b_o


# revision 44
# speedup vs baseline: 1.0501x; 1.0501x over previous
"""Multi-head attention kernel for 8 Trainium2 NeuronCores.

Problem: B=4, S=2048, D=H=1024, NH=16 heads (head_dim 64), causal MHA with
input projections (W_q/W_k/W_v), softmax, and output projection (W_o).

Sharding: 8 cores = 4 batches x 2 head-groups (tensor parallel over heads).
Each core computes, for one batch b and one group g of 8 heads (4 head-pairs):
  QT = (x W_q[g].T + b_q).T   [feature, seq]  (b_k dropped: its effect on
  KT = (x W_k[g].T).T                          softmax is a per-query constant)
  V  = x W_v[g].T             [seq, feature]  (b_v folded on host)
  per head-pair hp, per 512-wide q chunk, per 128-wide k block:
    S.T = KT_h.T QT_h  (zero-padded K strips -> full 128-row matmuls)
    P.T = exp(S.T/8 + causal) for both strips in one ACT instruction
    O.T += V_aug_h.T @ P.T  (ones column -> row 64 = softmax denominator)
  All score/exp/PV work is trimmed to the causal region at column granularity.
  partial.T = W_o[:, g].T-slice contracted with (O.T / denom)  (row-parallel)
The host sums the two group partials per batch, transposes, and adds
(W_o @ b_v + b_o) exactly.

Everything runs in bf16 operands (f32 PSUM accumulation): rel-err budget is
2e-2 and bf16 measures 4.8e-3 on HW. Scheduling: the V and Q/K projections
for later head-pairs are emitted as generators interleaved into the earlier
head-pairs' attention loops (and the output projection into the last one),
so the PE's projection matmuls fill the per-block gap left while the ACT
engine (exp) paces attention; causal trimming at column granularity plus the
0/1-triangle mask applied to P on the otherwise-idle GPSIMD engine keeps
ACT/DVE off the critical path.
"""

import os
import sys

if "/opt/trn_rl_repo" not in sys.path:
    sys.path.insert(0, "/opt/trn_rl_repo")

import numpy as np
import ml_dtypes

import concourse.mybir as mybir
import concourse.tile as tile
from concourse import bacc
from concourse.bass_utils import run_bass_kernel_spmd

F32 = mybir.dt.float32
BF16 = mybir.dt.bfloat16
F8 = mybir.dt.float8e4
EXP = mybir.ActivationFunctionType.Exp
CPY = mybir.ActivationFunctionType.Copy
IDN = mybir.ActivationFunctionType.Identity
DR = mybir.MatmulPerfMode.DoubleRow

# fp8 (DoubleRow, 2x PE) Q/K projections: measured rel err 3.5e-2 on HW —
# over the 2e-2 gate, so OFF. (bf16 everywhere measures 4.8e-3.)
QK_FP8 = os.environ.get("QK_FP8", "0") == "1"
WSCALE = 64.0

# Problem dims (full) and per-core dims
B, S, D, H, NH, HD = 4, 2048, 1024, 1024, 16, 64
HL = H // 2          # per-core feature width (8 heads x 64)
NHL = HL // HD       # 8 local heads
D_TILES = D // 128   # 8
J_TILES = HL // 128  # 4 head-pairs
S_TILES = S // 128   # 16
QC = S // 512        # 4 q-chunks (512 wide)
NEG = -1.0e30
PIPE = 8             # PV matmuls trail the score/exp pipeline by this many
QK_PACE = 3          # blocks per head-pair-2/3 projection feeder step
# NOTE: pace >= 4 EMITS the jt2/3 projection chains after head-pair 2's
# score matmuls that read them -> stale-read race, NaN on HW. Pace 3
# keeps every write 8+ blocks ahead of its reader.

_nc_cache = {}


def _build_nc(upto="all", reps=1):
    key = (upto, reps, QK_FP8)
    if key in _nc_cache:
        return _nc_cache[key]

    nc = bacc.Bacc("TRN2", target_bir_lowering=False, debug=False)

    QKDT = F8 if QK_FP8 else BF16
    xq_t = nc.dram_tensor("xq_t", [D, S], QKDT, kind="ExternalInput")
    xk_t = nc.dram_tensor("xk_t", [D, S], QKDT, kind="ExternalInput")
    xv_t = nc.dram_tensor("xv_t", [D, S], BF16, kind="ExternalInput")
    wq_t = nc.dram_tensor("wq_t", [D, HL], QKDT, kind="ExternalInput")
    wk_t = nc.dram_tensor("wk_t", [D, HL], QKDT, kind="ExternalInput")
    wv_t = nc.dram_tensor("wv_t", [D, HL], BF16, kind="ExternalInput")
    wo_t = nc.dram_tensor("wo_t", [HL, D], BF16, kind="ExternalInput")
    bq = nc.dram_tensor("bq", [HL], F32, kind="ExternalInput")
    masks = nc.dram_tensor("masks", [128, 2, 128], BF16, kind="ExternalInput")
    partial_t = nc.dram_tensor("partial_t", [D, S], BF16, kind="ExternalOutput")
    rr_dram = nc.dram_tensor("rr_scratch", [NHL, S], BF16, kind="Internal")

    with tile.TileContext(nc) as tc:
        with tc.tile_pool(name="persist", bufs=1) as pp:
            mask_sb = pp.tile([128, 2, 128], BF16, tag="masks")
            bq_sb = pp.tile([128, J_TILES], F32, tag="bq")
            wo_sb = pp.tile([128, J_TILES, D], BF16, tag="wo")

            QT = [pp.tile([128, S], BF16, tag=f"qt{j}", name=f"qt{j}")
                  for j in range(J_TILES)]
            # zero-padded K strip copies: KT[j][0] has head-strip 0 rows
            # (0:64) live and rows 64:128 zero, KT[j][1] the reverse, so
            # score matmuls contract the dead rows against zeros.
            KT = [[pp.tile([128, S], BF16, tag=f"kt{j}_{s}", name=f"kt{j}_{s}")
                   for s in range(2)] for j in range(J_TILES)]
            V = [pp.tile([128, NHL, HD + 1], BF16, tag=f"v{st}", name=f"v{st}")
                 for st in range(S_TILES)]
            OT = [pp.tile([128, S], BF16, tag=f"ot{j}", name=f"ot{j}")
                  for j in range(J_TILES)]
            RBD = pp.tile([128, S], BF16, tag="rbd")
            RB = pp.tile([128, S], F32, tag="rb")

            for _rep in range(reps):
                _emit_pipeline(nc, tc, upto, mask_sb, bq_sb, wo_sb, QT, KT, V,
                               OT, RB, RBD, rr_dram, xq_t, xk_t, xv_t, wq_t,
                               wk_t, wv_t, wo_t, partial_t, masks, bq)

    nc.finalize()
    _nc_cache[key] = nc
    return nc


def _emit_pipeline(nc, tc, upto, mask_sb, bq_sb, wo_sb, QT, KT, V, OT, RB,
                   RBD, rr_dram, xq_t, xk_t, xv_t, wq_t, wk_t, wv_t, wo_t,
                   partial_t, masks_d, bq_d):
    from contextlib import ExitStack

    # zero the dead K strip halves (idle GPSIMD engine)
    for j in range(J_TILES):
        nc.gpsimd.memset(KT[j][0][64:128, :], 0.0)
        nc.gpsimd.memset(KT[j][1][0:64, :], 0.0)
    for st in range(S_TILES):
        nc.gpsimd.memset(V[st][:, :, HD], 1.0)  # softmax-denominator column

    import itertools

    # attention pools open first (outermost) so the projection pools can be
    # released mid-stream (pool release must be LIFO)
    actx = ExitStack()
    with actx:
        scp = actx.enter_context(
            tc.tile_pool(name="scp", bufs=1, space="PSUM"))
        pvp = actx.enter_context(
            tc.tile_pool(name="pvp", bufs=1, space="PSUM"))
        ptp = actx.enter_context(tc.tile_pool(name="ptp", bufs=1))
        stg = actx.enter_context(tc.tile_pool(name="stg", bufs=1))
        _emit_inner(nc, tc, upto, mask_sb, bq_sb, wo_sb, QT, KT, V, OT, RB,
                    RBD, rr_dram, xq_t, xk_t, xv_t, wq_t, wk_t, wv_t, wo_t,
                    partial_t, scp, pvp, ptp, stg, masks_d, bq_d)


def _emit_inner(nc, tc, upto, mask_sb, bq_sb, wo_sb, QT, KT, V, OT, RB,
                RBD, rr_dram, xq_t, xk_t, xv_t, wq_t, wk_t, wv_t, wo_t,
                partial_t, scp, pvp, ptp, stg, masks_d, bq_d):
    from contextlib import ExitStack
    import itertools

    ctx = ExitStack()
    with ctx:
        qkw = ctx.enter_context(tc.tile_pool(name="qkw", bufs=1))
        qkx = ctx.enter_context(tc.tile_pool(name="qkx", bufs=1))
        # shared by the V-projection and Q/K-projection chains
        qkp = ctx.enter_context(tc.tile_pool(name="qkp", bufs=2, space="PSUM"))

        wv_sb = qkw.tile([128, D_TILES, HL], BF16, tag="wv", name="wv")
        wv_loaded = [False]

        def v_feeder(st_from, st_to, on_act=False):
            for st in range(st_from, st_to):
                ssl = slice(st * 128, (st + 1) * 128)
                xv_blk = qkx.tile([128, D_TILES, 128], BF16, tag="xvb",
                                  name="xvb", bufs=3)
                nc.sync.dma_start(
                    out=xv_blk,
                    in_=xv_t[:, ssl].rearrange("(dt p) f -> p dt f", p=128),
                )
                if not wv_loaded[0]:
                    # issue behind the first xv block; split so the chain's
                    # first matmuls only wait for the first half
                    wv_loaded[0] = True
                    for h in range(2):
                        nc.sync.dma_start(
                            out=wv_sb[:, 4 * h:4 * h + 4, :],
                            in_=wv_t[512 * h:512 * h + 512, :]
                            .rearrange("(dt p) j -> p dt j", p=128),
                        )
                yield
                ps = qkp.tile([128, HL], F32, tag="pp")
                for dt in range(D_TILES):
                    nc.tensor.matmul(
                        ps, xv_blk[:, dt, :], wv_sb[:, dt, :],
                        start=(dt == 0), stop=(dt == D_TILES - 1),
                    )
                if on_act:
                    nc.scalar.activation(
                        V[st][:, :, 0:HD],
                        ps[:].rearrange("p (h x) -> p h x", h=NHL), CPY)
                else:
                    nc.vector.tensor_copy(
                        V[st][:, :, 0:HD],
                        ps[:].rearrange("p (h x) -> p h x", h=NHL)
                    )
                yield

        def qk_feeder(jts, on_act=False):
            """Emit the Q/K projections for the given head-pairs, yielding
            between chunks so attention emission can interleave."""
            jw = 128 * len(jts)
            jsl_dram = slice(jts[0] * 128, jts[0] * 128 + jw)
            qkdt = F8 if QK_FP8 else BF16
            for (w_dram, x_dram, is_k, wtag) in (
                (wq_t, xq_t, False, "wq"),
                (wk_t, xk_t, True, "wk"),
            ):
                w_sb = qkw.tile([128, D_TILES, jw], qkdt,
                                tag=f"{wtag}{len(jts)}", name=wtag, bufs=2)
                nc.sync.dma_start(
                    out=w_sb,
                    in_=w_dram[:, jsl_dram].rearrange("(dt p) j -> p dt j",
                                                      p=128),
                )
                for sc in range(4):
                    scsl = slice(sc * 512, (sc + 1) * 512)
                    xblks = []
                    for half in range(2):
                        xb = qkx.tile([128, 4, 512], qkdt, tag=f"xb{half}",
                                      name=f"xb{half}", bufs=2)
                        nc.sync.dma_start(
                            out=xb,
                            in_=x_dram[half * 512:(half + 1) * 512, scsl]
                            .rearrange("(dt p) f -> p dt f", p=128),
                        )
                        xblks.append(xb)
                    yield
                    for jloc, jt in enumerate(jts):
                        jsl = slice(jloc * 128, (jloc + 1) * 128)
                        ps = qkp.tile([128, 512], F32, tag="pp")
                        if QK_FP8:
                            for i in range(4):  # dt pairs, DoubleRow
                                nc.tensor.matmul(
                                    ps, w_sb[:, 2 * i:2 * i + 2, jsl],
                                    xblks[i // 2][:, 2 * (i % 2):
                                                  2 * (i % 2) + 2, :],
                                    start=(i == 0), stop=(i == 3),
                                    perf_mode=DR,
                                )
                        else:
                            for dt in range(D_TILES):
                                nc.tensor.matmul(
                                    ps, w_sb[:, dt, jsl],
                                    xblks[dt // 4][:, dt % 4, :],
                                    start=(dt == 0), stop=(dt == D_TILES - 1),
                                )
                        if is_k:
                            if on_act:
                                nc.scalar.activation(KT[jt][0][0:64, scsl],
                                                     ps[0:64, :], CPY)
                                nc.scalar.activation(KT[jt][1][64:128, scsl],
                                                     ps[64:128, :], CPY)
                            else:
                                nc.vector.tensor_copy(KT[jt][0][0:64, scsl],
                                                      ps[0:64, :])
                                nc.vector.tensor_copy(KT[jt][1][64:128, scsl],
                                                      ps[64:128, :])
                        elif on_act:
                            nc.scalar.activation(QT[jt][:, scsl], ps, IDN,
                                                 bias=bq_sb[:, jt:jt + 1])
                        else:
                            nc.vector.tensor_scalar_add(
                                QT[jt][:, scsl], ps, bq_sb[:, jt:jt + 1]
                            )
                        yield

        # prologue: V tiles 0..3, then Q/K for head-pair 0 only (PSUM
        # drains on the ACT engine, which would otherwise idle until
        # attention); head-pair 1's projections stream into the attention
        # loop with everything else
        for _ in v_feeder(0, 4, on_act=True):
            pass
        # consts issue behind the first V loads: SP serializes DMA issue at
        # ~650ns each, and these aren't needed until attention starts
        nc.sync.dma_start(out=mask_sb, in_=masks_d[:])
        nc.sync.dma_start(
            out=bq_sb, in_=bq_d[:].rearrange("(t p) -> p t", p=128))
        for _ in qk_feeder((0, 1), on_act=True):
            pass
        nc.sync.dma_start(
            out=wo_sb, in_=wo_t[:].rearrange("(jt p) d -> p jt d", p=128)
        )
        if upto == "proj":
            for _ in v_feeder(4, S_TILES):
                pass
            for _ in qk_feeder((2, 3)):
                pass
            return

        # ================= attention =================
        def attention(hp, feeder, qi_end=None, qis=None):
            for Qi in (range(QC) if qis is None else qis):
                q0 = Qi * 512
                qsl = slice(q0, q0 + 512)
                nk = 4 * (Qi + 1)
                pv = [pvp.tile([HD + 1, 512], F32, tag=f"pv{s}",
                               name=f"pv{s}") for s in range(2)]
                pending = []

                def flush_one():
                    pt_, off_, ki_ = pending.pop(0)
                    for s in range(2):
                        nc.tensor.matmul(
                            pv[s][:, off_:512], V[ki_][:, 2 * hp + s, :],
                            pt_[:, s, off_:512],
                            start=(ki_ == 0), stop=(ki_ == nk - 1),
                            skip_group_check=True,
                        )

                for ki in range(nk):
                    k0 = ki * 128
                    off = max(0, k0 - q0)
                    st = scp.tile([128, 2, 512], F32, tag="st",
                                  name="st", bufs=2)
                    for s in range(2):
                        nc.tensor.matmul(
                            st[:, s, off:512], KT[hp][s][:, k0:k0 + 128],
                            QT[hp][:, q0 + off:q0 + 512],
                            start=True, stop=True,
                        )
                    pt = ptp.tile([128, 2, 512], BF16, tag="pt",
                                  name="pt", bufs=PIPE + 2)
                    escale = 0.125 / (WSCALE * WSCALE) if QK_FP8 else 0.125
                    nc.scalar.activation(pt[:, :, off:512],
                                         st[:, :, off:512], EXP,
                                         scale=escale)
                    if k0 >= q0:  # diagonal block: 0/1 triangle on idle Q7
                        nc.gpsimd.tensor_mul(pt[:, :, off:off + 128],
                                             pt[:, :, off:off + 128],
                                             mask_sb)
                    pending.append((pt, off, ki))
                    if len(pending) > PIPE:
                        flush_one()
                    if feeder is not None:
                        next(feeder, None)
                while pending:
                    flush_one()
                for s in range(2):
                    stage = stg.tile([HD + 1, 512], BF16, tag="stg",
                                     name="stage", bufs=3)
                    nc.vector.tensor_copy(stage, pv[s][:, :])
                    nc.sync.dma_start(
                        out=OT[hp][s * 64:(s + 1) * 64, qsl],
                        in_=stage[0:64, :])
                    # denominator row round-trips through DRAM so the DMA
                    # engine partition-broadcasts it into RBD
                    nc.sync.dma_start(
                        out=rr_dram[2 * hp + s, qsl], in_=stage[64:65, :])
                    nc.sync.dma_start(
                        out=RBD[s * 64:(s + 1) * 64, qsl],
                        in_=rr_dram[2 * hp + s:2 * hp + s + 1, qsl]
                        .to_broadcast((64, 512)))
                nc.vector.reciprocal(RB[:, qsl], RBD[:, qsl])
                nc.vector.tensor_mul(OT[hp][:, qsl], OT[hp][:, qsl],
                                     RB[:, qsl])
                if qi_end is not None:
                    qi_end(Qi)

        def paced(gen, period):
            for item in gen:
                yield item
                for _ in range(period - 1):
                    yield None

        # pace the later projections so they also soak up head-pair 2's
        # PE slack
        feeder = itertools.chain(v_feeder(4, S_TILES),
                                 paced(qk_feeder((2, 3)), QK_PACE))
        attention(0, feeder)
        attention(1, feeder)
        attention(2, feeder)
        for _ in feeder:  # make sure wave-1 projections are fully emitted
            pass
    # qk pools released here (SBUF + 2 PSUM banks)

    with (
        tc.tile_pool(name="opp", bufs=2, space="PSUM") as opp,
        tc.tile_pool(name="ost", bufs=1) as ost,
    ):
        def outproj_sc(sc):
            scsl = slice(sc * 512, (sc + 1) * 512)
            for dt in range(D_TILES):
                dsl = slice(dt * 128, (dt + 1) * 128)
                ps = opp.tile([128, 512], F32, tag="op")
                for jt in range(J_TILES):
                    nc.tensor.matmul(
                        ps, wo_sb[:, jt, dsl], OT[jt][:, scsl],
                        start=(jt == 0), stop=(jt == J_TILES - 1),
                    )
                ostage = ost.tile([128, 512], BF16, tag="ostage",
                                  name="ostage", bufs=3)
                nc.vector.tensor_copy(ostage, ps)
                nc.sync.dma_start(out=partial_t[dsl, scsl], in_=ostage)
                yield

        # interleave the output projection of q-chunk Qi-1 into head-pair
        # 3's attention on q-chunk Qi
        oproj = [None]

        def hp3_qi_end(Qi):
            if oproj[0] is not None:
                for _ in oproj[0]:
                    pass
            oproj[0] = outproj_sc(Qi)

        def hp3_feeder():
            while True:
                if oproj[0] is not None:
                    next(oproj[0], None)
                yield

        # descending q-chunks: the final chunk (Qi=0) is the shortest, so
        # the last normalize -> outproj tail is as small as possible
        attention(3, hp3_feeder(), qi_end=hp3_qi_end, qis=(3, 2, 1, 0))
        for _ in oproj[0]:
            pass


def _make_masks():
    p = np.arange(128)[:, None]
    f = np.arange(128)[None, :]
    tri = np.where(p > f, 0.0, 1.0).astype(ml_dtypes.bfloat16)
    return np.broadcast_to(tri[:, None, :], (128, 2, 128)).copy()


def _make_in_maps(q, k, v, W_q, b_q, W_k, b_k, W_v, b_v, W_o):
    bf = ml_dtypes.bfloat16
    qkdt = ml_dtypes.float8_e4m3 if QK_FP8 else bf
    ws = WSCALE if QK_FP8 else 1.0
    masks = _make_masks()
    in_maps = []
    for core in range(8):
        b, g = divmod(core, 2)
        gsl = slice(g * HL, (g + 1) * HL)
        in_maps.append({
            "xq_t": np.ascontiguousarray(q[b].T).astype(qkdt),
            "xk_t": np.ascontiguousarray(k[b].T).astype(qkdt),
            "xv_t": np.ascontiguousarray(v[b].T).astype(bf),
            "wq_t": np.ascontiguousarray(W_q[gsl].T * ws).astype(qkdt),
            "wk_t": np.ascontiguousarray(W_k[gsl].T * ws).astype(qkdt),
            "wv_t": np.ascontiguousarray(W_v[gsl].T).astype(bf),
            "wo_t": np.ascontiguousarray(W_o[:, gsl].T).astype(bf),
            "bq": np.ascontiguousarray(b_q[gsl] * ws).astype(np.float32),
            "masks": masks,
        })
    return in_maps


def kernel(q, k, v, padding_mask, W_q, b_q, W_k, b_k, W_v, b_v, W_o, b_o):
    q = np.asarray(q, np.float32)
    k = np.asarray(k, np.float32)
    v = np.asarray(v, np.float32)
    W_q = np.asarray(W_q, np.float32)
    W_k = np.asarray(W_k, np.float32)
    W_v = np.asarray(W_v, np.float32)
    W_o = np.asarray(W_o, np.float32)
    b_q = np.asarray(b_q, np.float32)
    b_k = np.asarray(b_k, np.float32)
    b_v = np.asarray(b_v, np.float32)
    b_o = np.asarray(b_o, np.float32)
    padding_mask = np.asarray(padding_mask)

    if padding_mask.any():
        return _numpy_reference(q, k, v, padding_mask, W_q, b_q, W_k, b_k,
                                W_v, b_v, W_o, b_o)

    nc = _build_nc()
    in_maps = _make_in_maps(q, k, v, W_q, b_q, W_k, b_k, W_v, b_v, W_o)

    res = run_bass_kernel_spmd(nc, in_maps, core_ids=list(range(8)))

    bias_vec = (W_o @ b_v + b_o).astype(np.float32)  # exact v/out bias folding
    out = np.empty((B, S, D), np.float32)
    for b in range(B):
        pt = (res.results[2 * b]["partial_t"].astype(np.float32)
              + res.results[2 * b + 1]["partial_t"].astype(np.float32))
        out[b] = pt.T + bias_vec
    return out


def _numpy_reference(q, k, v, padding_mask, W_q, b_q, W_k, b_k, W_v, b_v,
                     W_o, b_o):
    """Slow exact path, only used when padding_mask is nonzero."""
    Q = (q @ W_q.T + b_q).reshape(B, S, NH, HD).transpose(0, 2, 1, 3)
    K = (k @ W_k.T + b_k).reshape(B, S, NH, HD).transpose(0, 2, 1, 3)
    Vv = (v @ W_v.T + b_v).reshape(B, S, NH, HD).transpose(0, 2, 1, 3)
    scores = np.einsum("bhqd,bhkd->bhqk", Q, K) / np.sqrt(HD)
    causal = np.triu(np.ones((S, S), bool), k=1)
    scores = np.where(causal, -np.inf, scores)
    scores = np.where(padding_mask[:, None, None, :], -np.inf, scores)
    scores = scores - scores.max(axis=-1, keepdims=True)
    e = np.exp(scores)
    attn = e / e.sum(axis=-1, keepdims=True)
    out = np.einsum("bhqk,bhkd->bhqd", attn, Vv)
    out = out.transpose(0, 2, 1, 3).reshape(B, S, H)
    return out @ W_o.T + b_o
